# revision 34
# baseline (speedup 1.0000x reference)
"""Trainium2 Bass kernel: neural CDE (reversible Heun scan) — data-parallel over 8 cores.

Strategy (per core, batch shard B=32):
  - Feature-on-partition layout [feat, B] for the whole MLP chain.
  - lipswish folded into next-layer weights (0.909 * W), hidden act = Silu.
  - t-input of vf/cvf folded into per-step bias tables (dt == 1, t = step index).
  - cvf last layer reordered c-major into 4 blocks of 128 output features;
    its bias added via a small accumulating matmul; tanh as one [128,128] ACT.
  - einsum('bhc,bc->bh', g, dx): per-step dx broadcast tile built by one
    parity-matrix matmul from a pre-scattered table, elementwise multiply on
    the vector engine, then 4 accumulating selection matmuls (+identity
    matmul adding f) reduce over c directly in PSUM.
  - Scan runs in a For_i hardware loop, 64 steps per iteration. All compute
    engines use only STATIC slices (register budget); per-step data (dx
    scatter blocks, bias-table columns, readout rows) moves through small
    double-buffered SBUF stages refilled by a few dynamic DMAs per iteration.
  - Readout (64->1) is one tiny matmul per step into a staged row, DMA'd out
    twice per iteration.
"""

import sys

import numpy as np

if "/opt/trn_rl_repo" not in sys.path:
    sys.path.insert(0, "/opt/trn_rl_repo")

B_FULL = 256
N_CORES = 8
B = B_FULL // N_CORES  # 32
H = 64
C = 8
WID = 128
NS = 2048
NT = NS + 1
U = 64  # steps per For_i iteration (two 32-step half-stages)
SC = 0.909

_PROG_CACHE = {}


def _build_program(ns):
    import concourse.bass as bass
    import concourse.mybir as mybir
    import concourse.tile as tile
    from concourse.bacc import Bacc
    from concourse.bass import ds

    f32 = mybir.dt.float32
    AF = mybir.ActivationFunctionType
    OP = mybir.AluOpType

    assert ns % U == 0
    dxh_cols = (ns + 33) * WID
    bt_cols = ns + U         # padded bias tables
    out_rows = ns + 32

    nc = Bacc()

    dr = {}
    for name, p, f in [
        ("initT", 8, B),
        ("Wi1T", 8, WID), ("bi1", WID, 1),
        ("Wi2T", WID, WID), ("bi2", WID, 1),
        ("Wi3T", WID, H), ("bi3", H, 1),
        ("Wv1T", H, WID),
        ("Wv2T", WID, WID), ("bv2", WID, 1),
        ("Wv3T", WID, H), ("bv3", H, 1),
        ("Wc1T", H, WID),
        ("Wc2T", WID, WID), ("bc2", WID, 1),
        ("Wc3R", WID, 512),
        ("b3cRT", 4, WID),
        ("ind4", 4, WID),
        ("parity", C, WID),
        ("S64", WID, H),
        ("I64", H, H),
        ("wr", H, 1),
        ("br", 1, 1),
    ]:
        dr[name] = nc.declare_dram_parameter(name, [p, f], f32, isOutput=False)
    btv1 = nc.declare_dram_parameter("btv1", [WID, bt_cols], f32, isOutput=False)
    btc1 = nc.declare_dram_parameter("btc1", [WID, bt_cols], f32, isOutput=False)
    dxh = nc.declare_dram_parameter("dxh", [C, dxh_cols], f32, isOutput=False)
    out_dram = nc.declare_dram_parameter("out", [1, out_rows * B], f32, isOutput=True)

    with tile.TileContext(nc) as tc:
        with (
            tc.tile_pool(name="sb", bufs=1) as sb,
            tc.tile_pool(name="ps", bufs=1, space=bass.MemorySpace.PSUM) as ps,
        ):
            # ---- persistent SBUF tiles ----
            s = {}
            for name, t in dr.items():
                p, f = t.shape
                s[name] = sb.tile([p, f], f32, name=f"sb_{name}", tag=name)
            yhat = sb.tile([H, B], f32, name="yhat")
            Bq = sb.tile([H, B], f32, name="Bq")
            r_s = sb.tile([H, B], f32, name="r_s")
            y1 = sb.tile([H, B], f32, name="y1")
            f1 = sb.tile([H, B], f32, name="f1")
            a1v = sb.tile([WID, B], f32, name="a1v")
            a1c = sb.tile([WID, B], f32, name="a1c")
            a2v = sb.tile([WID, B], f32, name="a2v")
            a2c = sb.tile([WID, B], f32, name="a2c")
            G = sb.tile([WID, 4 * B], f32, name="G")
            gm1 = sb.tile([WID, 4 * B], f32, name="gm1")
            gm0 = sb.tile([WID, 4 * B], f32, name="gm0")
            b0v = sb.tile([WID, 1], f32, name="b0v")
            b0c = sb.tile([WID, 1], f32, name="b0c")
            dxs0 = sb.tile([C, WID], f32, name="dxs0")
            # double-buffered half-stages (index 0 -> steps u<32, 1 -> u>=32)
            st_dx = [sb.tile([C, 32 * WID], f32, name=f"st_dx{h}") for h in range(2)]
            st_v = [sb.tile([WID, 32], f32, name=f"st_v{h}") for h in range(2)]
            st_c = [sb.tile([WID, 32], f32, name=f"st_c{h}") for h in range(2)]
            st_o = [sb.tile([1, 32 * B], f32, name=f"st_o{h}") for h in range(2)]

            # ---- PSUM tiles (8 banks) ----
            pz1 = ps.tile([WID, 96], f32, name="pz1")   # Z1v | Z1c | Z3v(rows 0:64)
            pz2 = ps.tile([WID, 64], f32, name="pz2")   # Z2v | Z2c
            pG = ps.tile([WID, 4 * B], f32, name="pG")
            pdx = [ps.tile([WID, 4 * B], f32, name=f"pdx{i}") for i in range(2)]
            pP1 = ps.tile([WID, 64], f32, name="pP1")   # P1 rows 0:64 cols 0:32; ro [0:1,32:64]
            pP0 = [ps.tile([H, B], f32, name=f"pP0{i}") for i in range(2)]

            # ---- load constants / initial stages ----
            for name in dr:
                nc.sync.dma_start(out=s[name][:], in_=dr[name][:])
            nc.sync.dma_start(out=b0v[:], in_=btv1[:, 0:1])
            nc.sync.dma_start(out=b0c[:], in_=btc1[:, 0:1])
            nc.sync.dma_start(out=dxs0[:], in_=dxh[:, 0:WID])
            nc.sync.dma_start(out=st_dx[0][:], in_=dxh[:, WID:33 * WID])
            nc.sync.dma_start(out=st_v[0][:], in_=btv1[:, 1:33])
            nc.sync.dma_start(out=st_c[0][:], in_=btc1[:, 1:33])

            def mlp_step(src, bias_v, bias_c):
                """vf+cvf MLPs on `src` [H,B]. Leaves f1 (tanh vf) and
                G (tanh cvf, c-major blocks)."""
                nc.tensor.matmul(pz1[:, 0:B], s["Wv1T"][:], src[:], start=True, stop=True)
                nc.tensor.matmul(pz1[:, B:2 * B], s["Wc1T"][:], src[:], start=True, stop=True)
                nc.scalar.activation(a1v[:], pz1[:, 0:B], AF.Silu, bias=bias_v)
                nc.scalar.activation(a1c[:], pz1[:, B:2 * B], AF.Silu, bias=bias_c)
                nc.tensor.matmul(pz2[:, 0:B], s["Wv2T"][:], a1v[:], start=True, stop=True)
                nc.tensor.matmul(pz2[:, B:2 * B], s["Wc2T"][:], a1c[:], start=True, stop=True)
                nc.scalar.activation(a2v[:], pz2[:, 0:B], AF.Silu, bias=s["bv2"][:])
                nc.scalar.activation(a2c[:], pz2[:, B:2 * B], AF.Silu, bias=s["bc2"][:])
                nc.tensor.matmul(pz1[0:H, 2 * B:3 * B], s["Wv3T"][:], a2v[:], start=True, stop=True)
                for i in range(4):
                    nc.tensor.matmul(
                        pG[:, i * B:(i + 1) * B],
                        s["Wc3R"][:, i * WID:(i + 1) * WID],
                        a2c[:],
                        start=(i == 0), stop=False,
                    )
                nc.tensor.matmul(pG[:], s["b3cRT"][:], s["ind4"][:], start=False, stop=True)
                nc.scalar.activation(f1[:], pz1[0:H, 2 * B:3 * B], AF.Tanh, bias=s["bv3"][:])
                nc.scalar.activation(G[:], pG[:], AF.Tanh)

            def einsum_to(gm, pdst):
                """pdst[0:H,0:B] = sum_c G*dx (from gm) + f1, accumulated in PSUM."""
                for i in range(4):
                    nc.tensor.matmul(
                        pdst[0:H, 0:B], s["S64"][:], gm[:, i * B:(i + 1) * B],
                        start=(i == 0), stop=False,
                    )
                nc.tensor.matmul(pdst[0:H, 0:B], s["I64"][:], f1[:], start=False, stop=True)

            def readout(dst):
                nc.tensor.matmul(pP1[0:1, 32:64], s["wr"][:], y1[:], start=True, stop=True)
                nc.vector.tensor_scalar_add(dst, pP1[0:1, 32:64], s["br"][0:1, 0:1])

            # ================= prologue =================
            # initial MLP: init -> y0 (into y1 tile)
            nc.tensor.matmul(pz1[:, 0:B], s["Wi1T"][:], s["initT"][:], start=True, stop=True)
            nc.scalar.activation(a1v[:], pz1[:, 0:B], AF.Relu, bias=s["bi1"][:])
            nc.tensor.matmul(pz2[:, 0:B], s["Wi2T"][:], a1v[:], start=True, stop=True)
            nc.scalar.activation(a2v[:], pz2[:, 0:B], AF.Relu, bias=s["bi2"][:])
            nc.tensor.matmul(pz1[0:H, 2 * B:3 * B], s["Wi3T"][:], a2v[:], start=True, stop=True)
            nc.scalar.activation(y1[:], pz1[0:H, 2 * B:3 * B], AF.Identity, bias=s["bi3"][:])

            nc.vector.tensor_copy(yhat[:], y1[:])
            nc.vector.tensor_copy(Bq[:], y1[:])

            # step "-1": f0/G0 at t=0, p0 for step 0 into pP0[0], dxrep_0 into pdx[0]
            nc.tensor.matmul(pdx[0][:], s["parity"][:], dxs0[:], start=True, stop=True)
            mlp_step(yhat, b0v[:], b0c[:])
            nc.vector.tensor_mul(gm0[:], G[:], pdx[0][:])
            einsum_to(gm0, pP0[0])
            nc.vector.scalar_tensor_tensor(
                r_s[:], pP0[0][:], 0.5, y1[:], op0=OP.mult, op1=OP.add,
            )
            readout(st_o[0][0:1, 0:B])
            nc.sync.dma_start(out=out_dram[0:1, 0:B], in_=st_o[0][0:1, 0:B])

            # ================= scan loop =================
            engines = (
                mybir.EngineType.PE,
                mybir.EngineType.Activation,
                mybir.EngineType.DVE,
            )
            with tc.For_i(0, ns, U, hint_engines=engines) as iv:
                # refill the second-half stages (consumed at u >= 32 below)
                nc.sync.dma_start(
                    out=st_dx[1][:], in_=dxh[:, 33 * WID:][:, ds(iv * WID, 32 * WID)])
                nc.sync.dma_start(out=st_v[1][:], in_=btv1[:, 33:][:, ds(iv, 32)])
                nc.sync.dma_start(out=st_c[1][:], in_=btc1[:, 33:][:, ds(iv, 32)])
                for u in range(U):
                    h = u // 32
                    su = u % 32
                    pdx_cur = pdx[u % 2]
                    pdx_nxt = pdx[(u + 1) % 2]
                    pP0_cur = pP0[u % 2]
                    pP0_nxt = pP0[(u + 1) % 2]

                    # dx broadcast tile for step k+1 (off critical path)
                    nc.tensor.matmul(
                        pdx_nxt[:], s["parity"][:],
                        st_dx[h][:, su * WID:(su + 1) * WID],
                        start=True, stop=True,
                    )
                    # yhat_{k+1} = B + p0
                    nc.vector.tensor_add(yhat[:], Bq[:], pP0_cur[:])
                    mlp_step(yhat, st_v[h][:, su:su + 1], st_c[h][:, su:su + 1])
                    nc.vector.tensor_mul(gm1[:], G[:], pdx_cur[:])
                    nc.vector.tensor_mul(gm0[:], G[:], pdx_nxt[:])
                    einsum_to(gm1, pP1)       # p1 = f1 + ein(g1, dx_k)
                    einsum_to(gm0, pP0_nxt)   # p0' = f1 + ein(g1, dx_{k+1})
                    nc.vector.scalar_tensor_tensor(
                        y1[:], pP1[0:H, 0:B], 0.5, r_s[:], op0=OP.mult, op1=OP.add,
                    )
                    nc.vector.scalar_tensor_tensor(
                        Bq[:], y1[:], 2.0, yhat[:], op0=OP.mult, op1=OP.subtract,
                    )
                    nc.vector.scalar_tensor_tensor(
                        r_s[:], pP0_nxt[:], 0.5, y1[:], op0=OP.mult, op1=OP.add,
                    )
                    readout(st_o[h][0:1, su * B:(su + 1) * B])
                    if u == 31:
                        nc.sync.dma_start(
                            out=out_dram[:, B:][:, ds(iv * B, 32 * B)], in_=st_o[0][:],
                        )
                        # refill first-half stages for the NEXT body — emitted
                        # after all first-half consumers so the writes order
                        # after this body's reads (WAR), not before (RAW).
                        nc.sync.dma_start(
                            out=st_dx[0][:],
                            in_=dxh[:, 65 * WID:][:, ds(iv * WID, 32 * WID)])
                        nc.sync.dma_start(out=st_v[0][:], in_=btv1[:, 65:][:, ds(iv, 32)])
                        nc.sync.dma_start(out=st_c[0][:], in_=btc1[:, 65:][:, ds(iv, 32)])
                # rows iv+33 .. iv+64
                nc.sync.dma_start(
                    out=out_dram[:, 33 * B:][:, ds(iv * B, 32 * B)], in_=st_o[1][:])

    nc.finalize()  # run Bacc passes (wait-splitting, reg alloc) before compile
    return nc


def _get_prog(ns=NS):
    if ns not in _PROG_CACHE:
        _PROG_CACHE[ns] = _build_program(ns)
    return _PROG_CACHE[ns]


def _prep_core_inputs(ts, init, control, params, ns=NS):
    """Host-side preprocessing. Returns list of 8 per-core input dicts."""
    f = np.float32
    dxh_cols = (ns + 33) * WID
    bt_cols = ns + U

    def A(x):
        return np.ascontiguousarray(np.asarray(x, dtype=f))

    (Wi1, bi1), (Wi2, bi2), (Wi3, bi3) = [(A(w), A(b)) for w, b in params["initial"]]
    (Wv1, bv1), (Wv2, bv2), (Wv3, bv3) = [(A(w), A(b)) for w, b in params["vf"]]
    (Wc1, bc1), (Wc2, bc2), (Wc3, bc3) = [(A(w), A(b)) for w, b in params["cvf"]]
    Wr, br = [(A(w), A(b)) for w, b in params["readout"]][0]
    ts = A(ts)
    init = A(init)
    control = A(control)

    tt = np.zeros(bt_cols, f)
    tt[: ns + 1] = ts[: ns + 1]
    btv1 = np.zeros((WID, bt_cols), f)
    btv1[:, : ns + 1] = bv1[:, None] + np.outer(Wv1[:, 0], tt[: ns + 1])
    btc1 = np.zeros((WID, bt_cols), f)
    btc1[:, : ns + 1] = bc1[:, None] + np.outer(Wc1[:, 0], tt[: ns + 1])

    perm = np.array(
        [h * C + 2 * i + g for i in range(4) for g in range(2) for h in range(H)],
        dtype=np.int64,
    )
    common = {
        "Wi1T": A(Wi1.T), "bi1": A(bi1[:, None]),
        "Wi2T": A(Wi2.T), "bi2": A(bi2[:, None]),
        "Wi3T": A(Wi3.T), "bi3": A(bi3[:, None]),
        "Wv1T": A(Wv1[:, 1:].T), "btv1": A(btv1),
        "Wv2T": A((SC * Wv2).T), "bv2": A(bv2[:, None]),
        "Wv3T": A((SC * Wv3).T), "bv3": A(bv3[:, None]),
        "Wc1T": A(Wc1[:, 1:].T), "btc1": A(btc1),
        "Wc2T": A((SC * Wc2).T), "bc2": A(bc2[:, None]),
        "Wc3R": A((SC * Wc3).T[:, perm]),
        "b3cRT": A(bc3[perm].reshape(4, WID)),
        "ind4": A(np.kron(np.eye(4, dtype=f), np.ones((1, B), f))),
        "parity": A(np.fromfunction(
            lambda c, r: (c % 2 == r // H).astype(f), (C, WID))),
        "S64": A(np.concatenate([np.eye(H, dtype=f)] * 2, axis=0)),
        "I64": A(np.eye(H, dtype=f)),
        "wr": A(Wr.reshape(1, H).T),
        "br": A(br.reshape(1, 1)),
    }

    dX = control[:, 1:ns + 1] - control[:, :ns]  # [B_full, ns, C]
    in_maps = []
    for core in range(N_CORES):
        sl = slice(core * B, (core + 1) * B)
        dxc = dX[sl]  # [B, ns, C]
        # scatter: scat[j, c, 32*(c//2) + b] = dxc[b, j, c]
        scat = np.zeros((ns + 1, C, 4, B), f)
        for c in range(C):
            scat[:ns, c, c // 2, :] = dxc[:, :, c].T
        scat = scat.reshape(ns + 1, C, 4 * B)
        # dxh[c, j*128 + x] = scat[j, c, x] for j = 0..ns, zero-padded beyond
        dxh = np.zeros((C, dxh_cols), f)
        dxh[:, : (ns + 1) * WID] = scat.transpose(1, 0, 2).reshape(C, (ns + 1) * WID)
        m = dict(common)
        m["initT"] = A(init[sl].T)
        m["dxh"] = A(dxh)
        in_maps.append(m)
    return in_maps


def kernel(ts, init, control, params):
    from concourse import bass_utils

    nc = _get_prog(NS)
    in_maps = _prep_core_inputs(ts, init, control, params, NS)
    res = bass_utils.run_bass_kernel_spmd(nc, in_maps, core_ids=list(range(N_CORES)))
    outs = []
    for i in range(N_CORES):
        o = np.asarray(res.results[i]["out"]).reshape(-1, B)  # [ns+32, B]
        outs.append(o[:NT])
    return np.concatenate(outs, axis=1)[..., None].astype(np.float32)


# revision 43
# speedup vs baseline: 1.0497x; 1.0497x over previous
"""Trainium2 Bass kernel: neural CDE (reversible Heun scan) — data-parallel over 8 cores.

Strategy (per core, batch shard B=32):
  - Feature-on-partition layout [feat, B] for the whole MLP chain.
  - lipswish folded into next-layer weights (0.909 * W), hidden act = Silu.
  - t-input of vf/cvf folded into per-step bias tables (dt == 1, t = step index).
  - cvf last layer reordered c-major into 4 blocks of 128 output features;
    its bias added via a small accumulating matmul; tanh as one [128,128] ACT.
  - einsum('bhc,bc->bh', g, dx): per-step dx broadcast tile built by one
    parity-matrix matmul from a pre-scattered table, elementwise multiply on
    the vector engine, then 4 accumulating selection matmuls (+identity
    matmul adding f) reduce over c directly in PSUM.
  - Scan runs in a For_i hardware loop, 64 steps per iteration. All compute
    engines use only STATIC slices (register budget); per-step data (dx
    scatter blocks, bias-table columns, readout rows) moves through small
    double-buffered SBUF stages refilled by a few dynamic DMAs per iteration.
  - Readout (64->1) is one tiny matmul per step into a staged row, DMA'd out
    twice per iteration.
"""

import sys

import numpy as np

if "/opt/trn_rl_repo" not in sys.path:
    sys.path.insert(0, "/opt/trn_rl_repo")

B_FULL = 256
N_CORES = 8
B = B_FULL // N_CORES  # 32
H = 64
C = 8
WID = 128
NS = 2048
NT = NS + 1
U = 64  # steps per For_i iteration (two 32-step half-stages)
SC = 0.909

_PROG_CACHE = {}


def _build_program(ns):
    import concourse.bass as bass
    import concourse.mybir as mybir
    import concourse.tile as tile
    from concourse.bacc import Bacc
    from concourse.bass import ds

    f32 = mybir.dt.float32
    f32r = mybir.dt.float32r  # single-pass PE matmul (fp32 is split into 2)
    AF = mybir.ActivationFunctionType
    OP = mybir.AluOpType

    assert ns % U == 0
    dxh_cols = (ns + 33) * WID
    bt_cols = ns + U         # padded bias tables
    out_rows = ns + 32

    nc = Bacc()

    dr = {}
    for name, p, f in [
        ("initT", 8, B),
        ("Wi1T", 8, WID), ("bi1", WID, 1),
        ("Wi2T", WID, WID), ("bi2", WID, 1),
        ("Wi3T", WID, H), ("bi3", H, 1),
        ("Wv1T", H, WID),
        ("Wv2T", WID, WID), ("bv2", WID, 1),
        ("Wv3T", WID, H), ("bv3", H, 1),
        ("Wc1T", H, WID),
        ("Wc2T", WID, WID), ("bc2", WID, 1),
        ("Wc3R", WID, 512),
        ("b3cRT", 4, WID),
        ("ind4", 4, WID),
        ("parity", C, WID),
        ("S64", WID, H),
        ("I64", H, H),
        ("wr", H, 1),
        ("br", 1, 1),
    ]:
        dr[name] = nc.declare_dram_parameter(name, [p, f], f32, isOutput=False)
    btv1 = nc.declare_dram_parameter("btv1", [WID, bt_cols], f32, isOutput=False)
    btc1 = nc.declare_dram_parameter("btc1", [WID, bt_cols], f32, isOutput=False)
    dxh = nc.declare_dram_parameter("dxh", [C, dxh_cols], f32, isOutput=False)
    out_dram = nc.declare_dram_parameter("out", [1, out_rows * B], f32, isOutput=True)

    with tile.TileContext(nc) as tc:
        with (
            tc.tile_pool(name="sb", bufs=1) as sb,
            tc.tile_pool(name="ps", bufs=1, space=bass.MemorySpace.PSUM) as ps,
        ):
            # ---- persistent SBUF tiles ----
            s = {}
            for name, t in dr.items():
                p, f = t.shape
                s[name] = sb.tile([p, f], t.dtype, name=f"sb_{name}", tag=name)
            yhat = sb.tile([H, B], f32, name="yhat")
            Bq = sb.tile([H, B], f32, name="Bq")
            r_s = sb.tile([H, B], f32, name="r_s")
            y1 = sb.tile([H, B], f32, name="y1")
            f1 = sb.tile([H, B], f32, name="f1")
            p1s = sb.tile([H, B], f32, name="p1s")
            p0s = sb.tile([H, B], f32, name="p0s")
            a1v = sb.tile([WID, B], f32, name="a1v")
            a1c = sb.tile([WID, B], f32, name="a1c")
            a2v = sb.tile([WID, B], f32, name="a2v")
            a2c = sb.tile([WID, B], f32, name="a2c")
            G = sb.tile([WID, 4 * B], f32, name="G")
            gm1 = sb.tile([WID, 4 * B], f32, name="gm1")
            gm0 = sb.tile([WID, 4 * B], f32, name="gm0")
            b0v = sb.tile([WID, 1], f32, name="b0v")
            b0c = sb.tile([WID, 1], f32, name="b0c")
            dxs0 = sb.tile([C, WID], f32, name="dxs0")
            # double-buffered half-stages (index 0 -> steps u<32, 1 -> u>=32)
            st_dx = [sb.tile([C, 32 * WID], f32, name=f"st_dx{h}") for h in range(2)]
            st_v = [sb.tile([WID, 32], f32, name=f"st_v{h}") for h in range(2)]
            st_c = [sb.tile([WID, 32], f32, name=f"st_c{h}") for h in range(2)]
            st_o = [sb.tile([1, 32 * B], f32, name=f"st_o{h}") for h in range(2)]

            # ---- PSUM tiles (8 banks) ----
            pz1 = ps.tile([WID, 96], f32, name="pz1")   # Z1v | Z1c | Z3v(rows 0:64)
            pz2 = ps.tile([WID, 64], f32, name="pz2")   # Z2v | Z2c
            pG = ps.tile([WID, 4 * B], f32, name="pG")
            pdx = [ps.tile([WID, 4 * B], f32, name=f"pdx{i}") for i in range(2)]
            pP1 = ps.tile([WID, 64], f32, name="pP1")   # P1 rows 0:64 cols 0:32; ro [0:1,32:64]
            pP0 = [ps.tile([H, B], f32, name=f"pP0{i}") for i in range(2)]

            # ---- load constants / initial stages ----
            for name in dr:
                nc.sync.dma_start(out=s[name][:], in_=dr[name][:])
            nc.sync.dma_start(out=b0v[:], in_=btv1[:, 0:1])
            nc.sync.dma_start(out=b0c[:], in_=btc1[:, 0:1])
            nc.sync.dma_start(out=dxs0[:], in_=dxh[:, 0:WID])
            nc.sync.dma_start(out=st_dx[0][:], in_=dxh[:, WID:33 * WID])
            nc.sync.dma_start(out=st_v[0][:], in_=btv1[:, 1:33])
            nc.sync.dma_start(out=st_c[0][:], in_=btc1[:, 1:33])

            def mlp_step(src, bias_v, bias_c):
                """vf+cvf MLPs on `src` [H,B]. Leaves f1 (tanh vf) and
                G (tanh cvf, c-major blocks)."""
                nc.tensor.matmul(pz1[:, 0:B], s["Wv1T"][:], src[:], start=True, stop=True)
                nc.tensor.matmul(pz1[:, B:2 * B], s["Wc1T"][:], src[:], start=True, stop=True)
                nc.scalar.activation(a1v[:], pz1[:, 0:B], AF.Silu, bias=bias_v)
                nc.scalar.activation(a1c[:], pz1[:, B:2 * B], AF.Silu, bias=bias_c)
                nc.tensor.matmul(pz2[:, 0:B], s["Wv2T"][:], a1v[:], start=True, stop=True)
                nc.tensor.matmul(pz2[:, B:2 * B], s["Wc2T"][:], a1c[:], start=True, stop=True)
                nc.scalar.activation(a2v[:], pz2[:, 0:B], AF.Silu, bias=s["bv2"][:])
                nc.scalar.activation(a2c[:], pz2[:, B:2 * B], AF.Silu, bias=s["bc2"][:])
                nc.tensor.matmul(pz1[0:H, 2 * B:3 * B], s["Wv3T"][:], a2v[:], start=True, stop=True)
                for i in range(4):
                    nc.tensor.matmul(
                        pG[:, i * B:(i + 1) * B],
                        s["Wc3R"][:, i * WID:(i + 1) * WID],
                        a2c[:],
                        start=(i == 0), stop=False,
                    )
                nc.tensor.matmul(pG[:], s["b3cRT"][:], s["ind4"][:], start=False, stop=True)
                nc.scalar.activation(f1[:], pz1[0:H, 2 * B:3 * B], AF.Tanh, bias=s["bv3"][:])
                nc.scalar.activation(G[:], pG[:], AF.Tanh)

            def einsum_to(gm, pdst, dst):
                """dst = f1 + sum_c G*dx (from gm); reduce over c in PSUM,
                then one vector add folds in f1."""
                for i in range(4):
                    nc.tensor.matmul(
                        pdst[0:H, 0:B], s["S64"][:], gm[:, i * B:(i + 1) * B],
                        start=(i == 0), stop=(i == 3),
                    )
                nc.vector.tensor_add(dst[:], f1[:], pdst[0:H, 0:B])

            def readout(dst):
                nc.tensor.matmul(pP1[0:1, 32:64], s["wr"][:], y1[:], start=True, stop=True)
                nc.vector.tensor_scalar_add(dst, pP1[0:1, 32:64], s["br"][0:1, 0:1])

            # ================= prologue =================
            # initial MLP: init -> y0 (into y1 tile)
            nc.tensor.matmul(pz1[:, 0:B], s["Wi1T"][:], s["initT"][:], start=True, stop=True)
            nc.scalar.activation(a1v[:], pz1[:, 0:B], AF.Relu, bias=s["bi1"][:])
            nc.tensor.matmul(pz2[:, 0:B], s["Wi2T"][:], a1v[:], start=True, stop=True)
            nc.scalar.activation(a2v[:], pz2[:, 0:B], AF.Relu, bias=s["bi2"][:])
            nc.tensor.matmul(pz1[0:H, 2 * B:3 * B], s["Wi3T"][:], a2v[:], start=True, stop=True)
            nc.scalar.activation(y1[:], pz1[0:H, 2 * B:3 * B], AF.Identity, bias=s["bi3"][:])

            nc.vector.tensor_copy(yhat[:], y1[:])
            nc.vector.tensor_copy(Bq[:], y1[:])

            # step "-1": f0/G0 at t=0, p0 for step 0 into pP0[0], dxrep_0 into pdx[0]
            nc.tensor.matmul(pdx[0][:], s["parity"][:], dxs0[:], start=True, stop=True)
            mlp_step(yhat, b0v[:], b0c[:])
            nc.vector.tensor_mul(gm0[:], G[:], pdx[0][:])
            einsum_to(gm0, pP0[0], p0s)
            nc.vector.scalar_tensor_tensor(
                r_s[:], p0s[:], 0.5, y1[:], op0=OP.mult, op1=OP.add,
            )
            readout(st_o[0][0:1, 0:B])
            nc.sync.dma_start(out=out_dram[0:1, 0:B], in_=st_o[0][0:1, 0:B])

            # ================= scan loop =================
            engines = (
                mybir.EngineType.PE,
                mybir.EngineType.Activation,
                mybir.EngineType.DVE,
            )
            with tc.For_i(0, ns, U, hint_engines=engines) as iv:
                # refill the second-half stages (consumed at u >= 32 below)
                nc.sync.dma_start(
                    out=st_dx[1][:], in_=dxh[:, 33 * WID:][:, ds(iv * WID, 32 * WID)])
                nc.sync.dma_start(out=st_v[1][:], in_=btv1[:, 33:][:, ds(iv, 32)])
                nc.sync.dma_start(out=st_c[1][:], in_=btc1[:, 33:][:, ds(iv, 32)])
                for u in range(U):
                    h = u // 32
                    su = u % 32
                    pdx_cur = pdx[u % 2]
                    pdx_nxt = pdx[(u + 1) % 2]
                    pP0_cur = pP0[u % 2]
                    pP0_nxt = pP0[(u + 1) % 2]

                    # dx broadcast tile for step k+1 (off critical path)
                    nc.tensor.matmul(
                        pdx_nxt[:], s["parity"][:],
                        st_dx[h][:, su * WID:(su + 1) * WID],
                        start=True, stop=True,
                    )
                    # yhat_{k+1} = B + p0
                    nc.vector.tensor_add(yhat[:], Bq[:], p0s[:])
                    mlp_step(yhat, st_v[h][:, su:su + 1], st_c[h][:, su:su + 1])
                    nc.vector.tensor_mul(gm1[:], G[:], pdx_cur[:])
                    nc.vector.tensor_mul(gm0[:], G[:], pdx_nxt[:])
                    einsum_to(gm1, pP1, p1s)       # p1 = f1 + ein(g1, dx_k)
                    einsum_to(gm0, pP0_nxt, p0s)   # p0' = f1 + ein(g1, dx_{k+1})
                    nc.vector.scalar_tensor_tensor(
                        y1[:], p1s[:], 0.5, r_s[:], op0=OP.mult, op1=OP.add,
                    )
                    nc.vector.scalar_tensor_tensor(
                        Bq[:], y1[:], 2.0, yhat[:], op0=OP.mult, op1=OP.subtract,
                    )
                    nc.vector.scalar_tensor_tensor(
                        r_s[:], p0s[:], 0.5, y1[:], op0=OP.mult, op1=OP.add,
                    )
                    readout(st_o[h][0:1, su * B:(su + 1) * B])
                    if u == 31:
                        nc.sync.dma_start(
                            out=out_dram[:, B:][:, ds(iv * B, 32 * B)], in_=st_o[0][:],
                        )
                        # refill first-half stages for the NEXT body — emitted
                        # after all first-half consumers so the writes order
                        # after this body's reads (WAR), not before (RAW).
                        nc.sync.dma_start(
                            out=st_dx[0][:],
                            in_=dxh[:, 65 * WID:][:, ds(iv * WID, 32 * WID)])
                        nc.sync.dma_start(out=st_v[0][:], in_=btv1[:, 65:][:, ds(iv, 32)])
                        nc.sync.dma_start(out=st_c[0][:], in_=btc1[:, 65:][:, ds(iv, 32)])
                # rows iv+33 .. iv+64
                nc.sync.dma_start(
                    out=out_dram[:, 33 * B:][:, ds(iv * B, 32 * B)], in_=st_o[1][:])

    nc.finalize()  # run Bacc passes (wait-splitting, reg alloc) before compile
    return nc


def _get_prog(ns=NS):
    if ns not in _PROG_CACHE:
        _PROG_CACHE[ns] = _build_program(ns)
    return _PROG_CACHE[ns]


def _prep_core_inputs(ts, init, control, params, ns=NS):
    """Host-side preprocessing. Returns list of 8 per-core input dicts."""
    f = np.float32
    dxh_cols = (ns + 33) * WID
    bt_cols = ns + U

    def A(x):
        return np.ascontiguousarray(np.asarray(x, dtype=f))

    (Wi1, bi1), (Wi2, bi2), (Wi3, bi3) = [(A(w), A(b)) for w, b in params["initial"]]
    (Wv1, bv1), (Wv2, bv2), (Wv3, bv3) = [(A(w), A(b)) for w, b in params["vf"]]
    (Wc1, bc1), (Wc2, bc2), (Wc3, bc3) = [(A(w), A(b)) for w, b in params["cvf"]]
    Wr, br = [(A(w), A(b)) for w, b in params["readout"]][0]
    ts = A(ts)
    init = A(init)
    control = A(control)

    tt = np.zeros(bt_cols, f)
    tt[: ns + 1] = ts[: ns + 1]
    btv1 = np.zeros((WID, bt_cols), f)
    btv1[:, : ns + 1] = bv1[:, None] + np.outer(Wv1[:, 0], tt[: ns + 1])
    btc1 = np.zeros((WID, bt_cols), f)
    btc1[:, : ns + 1] = bc1[:, None] + np.outer(Wc1[:, 0], tt[: ns + 1])

    perm = np.array(
        [h * C + 2 * i + g for i in range(4) for g in range(2) for h in range(H)],
        dtype=np.int64,
    )
    common = {
        "Wi1T": A(Wi1.T), "bi1": A(bi1[:, None]),
        "Wi2T": A(Wi2.T), "bi2": A(bi2[:, None]),
        "Wi3T": A(Wi3.T), "bi3": A(bi3[:, None]),
        "Wv1T": A(Wv1[:, 1:].T), "btv1": A(btv1),
        "Wv2T": A((SC * Wv2).T), "bv2": A(bv2[:, None]),
        "Wv3T": A((SC * Wv3).T), "bv3": A(bv3[:, None]),
        "Wc1T": A(Wc1[:, 1:].T), "btc1": A(btc1),
        "Wc2T": A((SC * Wc2).T), "bc2": A(bc2[:, None]),
        "Wc3R": A((SC * Wc3).T[:, perm]),
        "b3cRT": A(bc3[perm].reshape(4, WID)),
        "ind4": A(np.kron(np.eye(4, dtype=f), np.ones((1, B), f))),
        "parity": A(np.fromfunction(
            lambda c, r: (c % 2 == r // H).astype(f), (C, WID))),
        "S64": A(np.concatenate([np.eye(H, dtype=f)] * 2, axis=0)),
        "I64": A(np.eye(H, dtype=f)),
        "wr": A(Wr.reshape(1, H).T),
        "br": A(br.reshape(1, 1)),
    }

    dX = control[:, 1:ns + 1] - control[:, :ns]  # [B_full, ns, C]
    in_maps = []
    for core in range(N_CORES):
        sl = slice(core * B, (core + 1) * B)
        dxc = dX[sl]  # [B, ns, C]
        # scatter: scat[j, c, 32*(c//2) + b] = dxc[b, j, c]
        scat = np.zeros((ns + 1, C, 4, B), f)
        for c in range(C):
            scat[:ns, c, c // 2, :] = dxc[:, :, c].T
        scat = scat.reshape(ns + 1, C, 4 * B)
        # dxh[c, j*128 + x] = scat[j, c, x] for j = 0..ns, zero-padded beyond
        dxh = np.zeros((C, dxh_cols), f)
        dxh[:, : (ns + 1) * WID] = scat.transpose(1, 0, 2).reshape(C, (ns + 1) * WID)
        m = dict(common)
        m["initT"] = A(init[sl].T)
        m["dxh"] = A(dxh)
        in_maps.append(m)
    return in_maps


def kernel(ts, init, control, params):
    from concourse import bass_utils

    nc = _get_prog(NS)
    in_maps = _prep_core_inputs(ts, init, control, params, NS)
    res = bass_utils.run_bass_kernel_spmd(nc, in_maps, core_ids=list(range(N_CORES)))
    outs = []
    for i in range(N_CORES):
        o = np.asarray(res.results[i]["out"]).reshape(-1, B)  # [ns+32, B]
        outs.append(o[:NT])
    return np.concatenate(outs, axis=1)[..., None].astype(np.float32)
